# revision 13
# baseline (speedup 1.0000x reference)
"""LSTM encoder (ragged sequences) on 8 Trainium2 NeuronCores.

Data-parallel: batch (1024) sharded 128 rows/core. Per core, per time step:
  gates[B,4H] = onehot(ids_t) @ G  +  h_{t-1} @ W_hh.T      (PE, fp32r)
     where G[v] = emb[v] @ W_ih.T + b_ih + b_hh  (precomputed on device)
  i,f,g,o = ACT sigmoid/tanh with per-row bias +-1e9 for rows past their
     length (freezes c via f=1, i=0 and zeroes output via o=0)
  c = f*c + i*g                                              (DVE)
  out_t = (o * dropout_mask) * tanh(c)                       (DVE)
  hT = PE-transpose(out_t)   -> next step's stationary operand
last_c = c after the loop (frozen at each row's length by the f-trick).
last_h = hs[b, len-1] gathered on host from the hs output.
"""

import numpy as np

B, T = 1024, 128
V, E, H = 64, 256, 512
G4 = 4 * H  # 2048
NCORES = 8
BL = B // NCORES  # 128
NEG = -1.0e9

_CACHE = {}


def _build_nc():
    import concourse.bacc as bacc
    import concourse.bass as bass
    import concourse.tile as tile
    from concourse import mybir
    from concourse.masks import make_identity

    f32 = mybir.dt.float32
    f32r = mybir.dt.float32r
    Sig = mybir.ActivationFunctionType.Sigmoid
    Tanh = mybir.ActivationFunctionType.Tanh
    mult = mybir.AluOpType.mult
    add = mybir.AluOpType.add

    nc = bacc.Bacc(
        "TRN2", target_bir_lowering=False, debug=False, num_devices=NCORES
    )

    # --- DRAM I/O (per-core shapes) ---
    d_onehot = nc.dram_tensor("onehot", [T, V, BL], f32r, kind="ExternalInput")
    d_whhT = nc.dram_tensor("whhT", [H, G4], f32r, kind="ExternalInput")
    d_wihT = nc.dram_tensor("wihT", [E, G4], f32r, kind="ExternalInput")
    d_embT = nc.dram_tensor("embT", [E, V], f32r, kind="ExternalInput")
    d_bias = nc.dram_tensor("bias2", [G4], f32, kind="ExternalInput")
    d_dmask = nc.dram_tensor("dmask", [BL, H], f32, kind="ExternalInput")
    d_bneg = nc.dram_tensor("bneg", [BL, T], f32, kind="ExternalInput")
    d_bpos = nc.dram_tensor("bpos", [BL, T], f32, kind="ExternalInput")
    d_hs = nc.dram_tensor("hs", [BL, T, H], f32, kind="ExternalOutput")
    d_lastc = nc.dram_tensor("lastc", [BL, H], f32, kind="ExternalOutput")

    GATES = ["f", "i", "g", "o"]  # completion order (f first for the c chain)
    COL = {"i": 0, "f": 1, "g": 2, "o": 3}  # torch gate order i,f,g,o in 4H

    def nsl(name):
        c = COL[name]
        return slice(c * H, (c + 1) * H)

    with tile.TileContext(nc) as tc:
        with (
            tc.tile_pool(name="singles", bufs=1) as singles,
            tc.tile_pool(name="gates", bufs=2) as gpool,
            tc.tile_pool(name="mid", bufs=2) as mid,
            tc.tile_pool(name="outs", bufs=3) as outs,
            tc.tile_pool(name="hT", bufs=2) as hTp,
            tc.tile_pool(name="ps_fig", bufs=2, space="PSUM") as ps_fig,
            tc.tile_pool(name="ps_o", bufs=1, space="PSUM") as ps_o,
            tc.tile_pool(name="ps_t", bufs=1, space="PSUM") as ps_t,
        ):
            # --- persistent SBUF ---
            oh_sb = singles.tile([V, T, BL], f32r)
            nc.sync.dma_start(out=oh_sb, in_=d_onehot[:].rearrange("t v b -> v t b"))
            whh_sb = singles.tile([128, 4, G4], f32r)
            nc.sync.dma_start(
                out=whh_sb, in_=d_whhT[:].rearrange("(k p) n -> p k n", p=128)
            )
            wih_sb = singles.tile([128, 2, G4], f32r)
            nc.sync.dma_start(
                out=wih_sb, in_=d_wihT[:].rearrange("(k p) n -> p k n", p=128)
            )
            embT_sb = singles.tile([128, 2, V], f32r)
            nc.sync.dma_start(
                out=embT_sb, in_=d_embT[:].rearrange("(k p) v -> p k v", p=128)
            )
            bias_sb = singles.tile([V, G4], f32)
            nc.sync.dma_start(
                out=bias_sb,
                in_=bass.AP(tensor=d_bias, offset=0, ap=[[0, V], [1, G4]]),
            )
            dmask_sb = singles.tile([BL, H], f32)
            nc.sync.dma_start(out=dmask_sb, in_=d_dmask[:])
            bneg_sb = singles.tile([BL, T], f32)
            nc.sync.dma_start(out=bneg_sb, in_=d_bneg[:])
            bpos_sb = singles.tile([BL, T], f32)
            nc.sync.dma_start(out=bpos_sb, in_=d_bpos[:])
            ident = singles.tile([128, 128], f32)
            make_identity(nc, ident)
            G_sb = singles.tile([V, G4], f32r)
            c_sb = singles.tile([BL, H], f32)
            nc.vector.memset(c_sb, 0.0)

            def ps_tile(name):
                if name == "o":
                    return ps_o.tile([128, H], f32, tag="ps_o", name="ps_o")
                return ps_fig.tile(
                    [128, H], f32, tag="ps_" + name, name="ps_" + name
                )

            # --- G table: G[v] = emb[v] @ W_ih.T + (b_ih + b_hh) ---
            for name in GATES:
                s = nsl(name)
                gp = ps_tile(name)
                for k in range(2):
                    nc.tensor.matmul(
                        gp[:V, :],
                        embT_sb[:, k, :],
                        wih_sb[:, k, s],
                        start=(k == 0),
                        stop=(k == 1),
                    )
                nc.vector.tensor_tensor(G_sb[:, s], gp[:V, :], bias_sb[:, s], op=add)

            # --- recurrence ---
            # Bank completion order o,f,i,g: om=o*dmask runs early, and the
            # critical c-chain hangs off the g bank (last). g is processed in
            # two 256-col halves so the tail pipelines; out_t halves are
            # PE-transposed per 128-chunk, and the next step's h-matmuls are
            # emitted k-blocked (k0,k1 first) so PE restarts as soon as the
            # first two hT chunks are copied.
            BANKS = ["o", "f", "i", "g"]
            HH = H // 2  # 256

            DUMMY_N = 5

            def emit_xpart(t, first_stop):
                tiles = {}
                for name in BANKS:
                    gp = ps_tile(name)
                    if name == "o" and not first_stop:
                        # HAM heaters: keep the PE array active through this
                        # step's tail so the clock gate stays at 8/8. They
                        # write the o bank, which the real x-part matmul
                        # below immediately clears (start=True); nothing
                        # reads them.
                        for _ in range(DUMMY_N):
                            nc.tensor.matmul(
                                gp,
                                oh_sb[:, t, :],
                                G_sb[:, nsl(name)],
                                start=True,
                                stop=True,
                                skip_group_check=True,
                            )
                    nc.tensor.matmul(
                        gp,
                        oh_sb[:, t, :],
                        G_sb[:, nsl(name)],
                        start=True,
                        stop=first_stop,
                    )
                    tiles[name] = gp
                return tiles

            def emit_hpart(ps, hT, ks):
                # one k-block: banks o,f,i full width, g in two halves
                for name in BANKS:
                    gp = ps[name]
                    s0 = nsl(name).start
                    for k in ks:
                        hk = hT[:, k * 128 : (k + 1) * 128]
                        if name == "g":
                            for hh in range(2):
                                nc.tensor.matmul(
                                    gp[:, hh * HH : (hh + 1) * HH],
                                    hk,
                                    whh_sb[:, k, s0 + hh * HH : s0 + (hh + 1) * HH],
                                    start=False,
                                    stop=(k == 3),
                                    skip_group_check=True,
                                )
                        else:
                            nc.tensor.matmul(
                                gp,
                                hk,
                                whh_sb[:, k, nsl(name)],
                                start=False,
                                stop=(k == 3),
                                skip_group_check=True,
                            )

            ps_cur = emit_xpart(0, True)
            hT_prev = None

            for t in range(T):
                # tp is allocated at step start: HAM-heater matmuls keep the
                # PE busy (and the clock ungated) while waiting for hT; the
                # real transposes later overwrite the same psum tile.
                tp = (
                    ps_t.tile([128, H], f32, tag="tp", name="tp")
                    if t + 1 < T
                    else None
                )
                if t > 0:
                    emit_hpart(ps_cur, hT_prev, (0, 1))
                    emit_hpart(ps_cur, hT_prev, (2, 3))

                ps_step = ps_cur
                bneg_c = bneg_sb[:, t : t + 1]
                bpos_c = bpos_sb[:, t : t + 1]

                o_sb = gpool.tile([BL, H], f32, tag="o")
                f_sb = gpool.tile([BL, H], f32, tag="f")
                i_sb = gpool.tile([BL, H], f32, tag="i")
                g_sb = gpool.tile([BL, H], f32, tag="g")
                nc.scalar.activation(o_sb, ps_step["o"], Sig, bias=bneg_c)
                nc.scalar.activation(f_sb, ps_step["f"], Sig, bias=bpos_c)
                nc.scalar.activation(i_sb, ps_step["i"], Sig, bias=bneg_c)
                # g halves as soon as each half of the bank is done
                nc.scalar.activation(
                    g_sb[:, :HH], ps_step["g"][:, :HH], Tanh
                )
                nc.scalar.activation(
                    g_sb[:, HH:], ps_step["g"][:, HH:], Tanh
                )

                # next step's x-part into the other psum buffers while this
                # step's tail is still running
                if t + 1 < T:
                    ps_cur = emit_xpart(t + 1, False)

                om = mid.tile([BL, H], f32, tag="om")
                fc = mid.tile([BL, H], f32, tag="fc")
                ig = mid.tile([BL, H], f32, tag="ig")
                tanhc = mid.tile([BL, H], f32, tag="tanhc")
                out_t = outs.tile([BL, H], f32, tag="out")
                nc.vector.tensor_tensor(om, o_sb, dmask_sb, op=mult)
                nc.vector.tensor_tensor(fc, f_sb, c_sb, op=mult)
                hT_new = (
                    hTp.tile([128, H], f32r, tag="hT", name="hT")
                    if t + 1 < T
                    else None
                )
                for hh in range(2):
                    s = slice(hh * HH, (hh + 1) * HH)
                    nc.vector.tensor_tensor(ig[:, s], i_sb[:, s], g_sb[:, s], op=mult)
                    nc.vector.tensor_tensor(c_sb[:, s], fc[:, s], ig[:, s], op=add)
                    nc.scalar.activation(tanhc[:, s], c_sb[:, s], Tanh)
                    nc.vector.tensor_tensor(out_t[:, s], om[:, s], tanhc[:, s], op=mult)
                    if t + 1 < T:
                        for kk in range(2):
                            k = hh * 2 + kk
                            ksl = slice(k * 128, (k + 1) * 128)
                            nc.tensor.transpose(tp[:, ksl], out_t[:, ksl], ident)
                            if kk == 0:
                                nc.scalar.copy(hT_new[:, ksl], tp[:, ksl])
                            else:
                                nc.vector.tensor_copy(hT_new[:, ksl], tp[:, ksl])

                nc.sync.dma_start(out=d_hs[:, t, :], in_=out_t)
                hT_prev = hT_new

            nc.sync.dma_start(out=d_lastc[:], in_=c_sb)

    nc.compile()
    return nc


def _host_prep(input_ids, emb, W_ih, W_hh, b_ih, b_hh, dropout_mask):
    ids = np.asarray(input_ids).astype(np.int64)
    is_eos = ids == 0
    has = is_eos.any(axis=1)
    first = np.argmax(is_eos, axis=1)
    lengths = np.where(has, first + 1, T).astype(np.int64)  # [B]
    active = (np.arange(T)[None, :] < lengths[:, None]).astype(np.float32)  # [B,T]
    inv = 1.0 - active

    onehot = (ids[:, :, None] == np.arange(V)[None, None, :]).astype(np.float32)
    # [B,T,V] -> per-core [T,V,BL]
    whhT = np.ascontiguousarray(np.asarray(W_hh).T.astype(np.float32))  # [H,4H]
    wihT = np.ascontiguousarray(np.asarray(W_ih).T.astype(np.float32))  # [E,4H]
    embT = np.ascontiguousarray(np.asarray(emb).T.astype(np.float32))  # [E,V]
    bias2 = (np.asarray(b_ih) + np.asarray(b_hh)).astype(np.float32)  # [4H]
    dmask = np.asarray(dropout_mask).astype(np.float32)  # [B,H]

    in_maps = []
    for c in range(NCORES):
        sl = slice(c * BL, (c + 1) * BL)
        in_maps.append(
            {
                "onehot": np.ascontiguousarray(onehot[sl].transpose(1, 2, 0)),
                "whhT": whhT,
                "wihT": wihT,
                "embT": embT,
                "bias2": bias2,
                "dmask": np.ascontiguousarray(dmask[sl]),
                "bneg": np.ascontiguousarray(NEG * inv[sl]),
                "bpos": np.ascontiguousarray(-NEG * inv[sl]),
            }
        )
    return in_maps, lengths


def kernel(input_ids, emb, W_ih, W_hh, b_ih, b_hh, dropout_mask):
    from concourse.bass_utils import run_bass_kernel_spmd

    in_maps, lengths = _host_prep(
        input_ids, emb, W_ih, W_hh, b_ih, b_hh, dropout_mask
    )
    if "nc" not in _CACHE:
        _CACHE["nc"] = _build_nc()
    res = run_bass_kernel_spmd(_CACHE["nc"], in_maps, list(range(NCORES))).results

    hs = np.concatenate([r["hs"] for r in res], axis=0)  # [B,T,H]
    last_c = np.concatenate([r["lastc"] for r in res], axis=0)  # [B,H]
    last_h = hs[np.arange(B), lengths - 1, :]  # [B,H]
    return hs, last_h, last_c


# revision 14
# speedup vs baseline: 1.0316x; 1.0316x over previous
"""LSTM encoder (ragged sequences) on 8 Trainium2 NeuronCores.

Data-parallel: batch (1024) sharded 128 rows/core. Per core, per time step:
  gates[B,4H] = onehot(ids_t) @ G  +  h_{t-1} @ W_hh.T      (PE, fp32r)
     where G[v] = emb[v] @ W_ih.T + b_ih + b_hh  (precomputed on device)
  i,f,g,o = ACT sigmoid/tanh with per-row bias +-1e9 for rows past their
     length (freezes c via f=1, i=0 and zeroes output via o=0)
  c = f*c + i*g                                              (DVE)
  out_t = (o * dropout_mask) * tanh(c)                       (DVE)
  hT = PE-transpose(out_t)   -> next step's stationary operand
last_c = c after the loop (frozen at each row's length by the f-trick).
last_h = hs[b, len-1] gathered on host from the hs output.
"""

import numpy as np

B, T = 1024, 128
V, E, H = 64, 256, 512
G4 = 4 * H  # 2048
NCORES = 8
BL = B // NCORES  # 128
NEG = -1.0e9

_CACHE = {}


def _build_nc():
    import concourse.bacc as bacc
    import concourse.bass as bass
    import concourse.tile as tile
    from concourse import mybir
    from concourse.masks import make_identity

    f32 = mybir.dt.float32
    f32r = mybir.dt.float32r
    Sig = mybir.ActivationFunctionType.Sigmoid
    Tanh = mybir.ActivationFunctionType.Tanh
    mult = mybir.AluOpType.mult
    add = mybir.AluOpType.add

    nc = bacc.Bacc(
        "TRN2", target_bir_lowering=False, debug=False, num_devices=NCORES
    )

    # --- DRAM I/O (per-core shapes) ---
    d_onehot = nc.dram_tensor("onehot", [T, V, BL], f32r, kind="ExternalInput")
    d_whhT = nc.dram_tensor("whhT", [H, G4], f32r, kind="ExternalInput")
    d_wihT = nc.dram_tensor("wihT", [E, G4], f32r, kind="ExternalInput")
    d_embT = nc.dram_tensor("embT", [E, V], f32r, kind="ExternalInput")
    d_bias = nc.dram_tensor("bias2", [G4], f32, kind="ExternalInput")
    d_dmask = nc.dram_tensor("dmask", [BL, H], f32, kind="ExternalInput")
    d_bneg = nc.dram_tensor("bneg", [BL, T], f32, kind="ExternalInput")
    d_bpos = nc.dram_tensor("bpos", [BL, T], f32, kind="ExternalInput")
    d_hs = nc.dram_tensor("hs", [BL, T, H], f32, kind="ExternalOutput")
    d_lastc = nc.dram_tensor("lastc", [BL, H], f32, kind="ExternalOutput")

    GATES = ["f", "i", "g", "o"]  # completion order (f first for the c chain)
    COL = {"i": 0, "f": 1, "g": 2, "o": 3}  # torch gate order i,f,g,o in 4H

    def nsl(name):
        c = COL[name]
        return slice(c * H, (c + 1) * H)

    with tile.TileContext(nc) as tc:
        with (
            tc.tile_pool(name="singles", bufs=1) as singles,
            tc.tile_pool(name="gates", bufs=2) as gpool,
            tc.tile_pool(name="mid", bufs=2) as mid,
            tc.tile_pool(name="outs", bufs=3) as outs,
            tc.tile_pool(name="hT", bufs=2) as hTp,
            tc.tile_pool(name="ps_fg", bufs=2, space="PSUM") as ps_fg,
            tc.tile_pool(name="ps_io", bufs=1, space="PSUM") as ps_io,
            tc.tile_pool(name="ps_t", bufs=2, space="PSUM") as ps_t,
        ):
            # --- persistent SBUF ---
            oh_sb = singles.tile([V, T, BL], f32r)
            nc.sync.dma_start(out=oh_sb, in_=d_onehot[:].rearrange("t v b -> v t b"))
            whh_sb = singles.tile([128, 4, G4], f32r)
            nc.sync.dma_start(
                out=whh_sb, in_=d_whhT[:].rearrange("(k p) n -> p k n", p=128)
            )
            wih_sb = singles.tile([128, 2, G4], f32r)
            nc.sync.dma_start(
                out=wih_sb, in_=d_wihT[:].rearrange("(k p) n -> p k n", p=128)
            )
            embT_sb = singles.tile([128, 2, V], f32r)
            nc.sync.dma_start(
                out=embT_sb, in_=d_embT[:].rearrange("(k p) v -> p k v", p=128)
            )
            bias_sb = singles.tile([V, G4], f32)
            nc.sync.dma_start(
                out=bias_sb,
                in_=bass.AP(tensor=d_bias, offset=0, ap=[[0, V], [1, G4]]),
            )
            dmask_sb = singles.tile([BL, H], f32)
            nc.sync.dma_start(out=dmask_sb, in_=d_dmask[:])
            bneg_sb = singles.tile([BL, T], f32)
            nc.sync.dma_start(out=bneg_sb, in_=d_bneg[:])
            bpos_sb = singles.tile([BL, T], f32)
            nc.sync.dma_start(out=bpos_sb, in_=d_bpos[:])
            ident = singles.tile([128, 128], f32)
            make_identity(nc, ident)
            G_sb = singles.tile([V, G4], f32r)
            c_sb = singles.tile([BL, H], f32)
            nc.vector.memset(c_sb, 0.0)

            def ps_tile(name):
                pool = ps_io if name in ("o", "i") else ps_fg
                return pool.tile([128, H], f32, tag="ps_" + name, name="ps_" + name)

            # --- G table: G[v] = emb[v] @ W_ih.T + (b_ih + b_hh) ---
            for name in GATES:
                s = nsl(name)
                gp = ps_tile(name)
                for k in range(2):
                    nc.tensor.matmul(
                        gp[:V, :],
                        embT_sb[:, k, :],
                        wih_sb[:, k, s],
                        start=(k == 0),
                        stop=(k == 1),
                    )
                nc.vector.tensor_tensor(G_sb[:, s], gp[:V, :], bias_sb[:, s], op=add)

            # --- recurrence ---
            # Bank completion order o,f,i,g: om=o*dmask runs early, and the
            # critical c-chain hangs off the g bank (last). g is processed in
            # two 256-col halves so the tail pipelines; out_t halves are
            # PE-transposed per 128-chunk, and the next step's h-matmuls are
            # emitted k-blocked (k0,k1 first) so PE restarts as soon as the
            # first two hT chunks are copied.
            BANKS = ["o", "f", "i", "g"]
            HH = H // 2  # 256

            HEAT_PRE = 8
            HEAT_POST = 4

            def emit_xpart(t, first_stop):
                tiles = {}
                for name in BANKS:
                    gp = ps_tile(name)
                    nc.tensor.matmul(
                        gp,
                        oh_sb[:, t, :],
                        G_sb[:, nsl(name)],
                        start=True,
                        stop=first_stop,
                    )
                    tiles[name] = gp
                return tiles

            def emit_hpart(ps, hT, ks):
                # one k-block: banks o,f,i full width, g in two halves
                for name in BANKS:
                    gp = ps[name]
                    s0 = nsl(name).start
                    for k in ks:
                        hk = hT[:, k * 128 : (k + 1) * 128]
                        if name == "g":
                            for hh in range(2):
                                nc.tensor.matmul(
                                    gp[:, hh * HH : (hh + 1) * HH],
                                    hk,
                                    whh_sb[:, k, s0 + hh * HH : s0 + (hh + 1) * HH],
                                    start=False,
                                    stop=(k == 3),
                                    skip_group_check=True,
                                )
                        else:
                            nc.tensor.matmul(
                                gp,
                                hk,
                                whh_sb[:, k, nsl(name)],
                                start=False,
                                stop=(k == 3),
                                skip_group_check=True,
                            )

            ps_cur = emit_xpart(0, True)
            hT_prev = None

            for t in range(T):
                if t > 0:
                    emit_hpart(ps_cur, hT_prev, (0, 1))
                    emit_hpart(ps_cur, hT_prev, (2, 3))

                ps_step = ps_cur
                bneg_c = bneg_sb[:, t : t + 1]
                bpos_c = bpos_sb[:, t : t + 1]

                o_sb = gpool.tile([BL, H], f32, tag="o")
                f_sb = gpool.tile([BL, H], f32, tag="f")
                i_sb = gpool.tile([BL, H], f32, tag="i")
                g_sb = gpool.tile([BL, H], f32, tag="g")
                nc.scalar.activation(o_sb, ps_step["o"], Sig, bias=bneg_c)
                nc.scalar.activation(f_sb, ps_step["f"], Sig, bias=bpos_c)
                nc.scalar.activation(i_sb, ps_step["i"], Sig, bias=bneg_c)
                # g halves as soon as each half of the bank is done
                nc.scalar.activation(
                    g_sb[:, :HH], ps_step["g"][:, :HH], Tanh
                )
                nc.scalar.activation(
                    g_sb[:, HH:], ps_step["g"][:, HH:], Tanh
                )

                # fine-grained HAM heaters: the PE chews these (into the
                # already-consumed o bank) while the DVE/ACT tail produces
                # out_t; nothing reads them and the next x-part clears the
                # bank with start=True.
                if t > 0:
                    for _ in range(HEAT_PRE):
                        nc.tensor.matmul(
                            ps_step["o"][:, 0:128],
                            oh_sb[:, t, :],
                            G_sb[:, 0:128],
                            start=True,
                            stop=True,
                            skip_group_check=True,
                        )

                om = mid.tile([BL, H], f32, tag="om")
                fc = mid.tile([BL, H], f32, tag="fc")
                ig = mid.tile([BL, H], f32, tag="ig")
                tanhc = mid.tile([BL, H], f32, tag="tanhc")
                out_t = outs.tile([BL, H], f32, tag="out")
                nc.vector.tensor_tensor(om, o_sb, dmask_sb, op=mult)
                nc.vector.tensor_tensor(fc, f_sb, c_sb, op=mult)
                hT_new = (
                    hTp.tile([128, H], f32r, tag="hT", name="hT")
                    if t + 1 < T
                    else None
                )
                for hh in range(2):
                    s = slice(hh * HH, (hh + 1) * HH)
                    nc.vector.tensor_tensor(ig[:, s], i_sb[:, s], g_sb[:, s], op=mult)
                    nc.vector.tensor_tensor(c_sb[:, s], fc[:, s], ig[:, s], op=add)
                    nc.scalar.activation(tanhc[:, s], c_sb[:, s], Tanh)
                    nc.vector.tensor_tensor(out_t[:, s], om[:, s], tanhc[:, s], op=mult)
                    if t + 1 < T:
                        for kk in range(2):
                            k = hh * 2 + kk
                            ksl = slice(k * 128, (k + 1) * 128)
                            tpc = ps_t.tile([128, 128], f32, tag="tp", name="tp")
                            nc.tensor.transpose(tpc, out_t[:, ksl], ident)
                            if kk == 0:
                                nc.scalar.copy(hT_new[:, ksl], tpc)
                            else:
                                nc.vector.tensor_copy(hT_new[:, ksl], tpc)

                if t + 1 < T:
                    for _ in range(HEAT_POST):
                        nc.tensor.matmul(
                            ps_step["o"],
                            oh_sb[:, t, :],
                            G_sb[:, 0:H],
                            start=True,
                            stop=True,
                            skip_group_check=True,
                        )
                    ps_cur = emit_xpart(t + 1, False)

                nc.sync.dma_start(out=d_hs[:, t, :], in_=out_t)
                hT_prev = hT_new

            nc.sync.dma_start(out=d_lastc[:], in_=c_sb)

    nc.compile()
    return nc


def _host_prep(input_ids, emb, W_ih, W_hh, b_ih, b_hh, dropout_mask):
    ids = np.asarray(input_ids).astype(np.int64)
    is_eos = ids == 0
    has = is_eos.any(axis=1)
    first = np.argmax(is_eos, axis=1)
    lengths = np.where(has, first + 1, T).astype(np.int64)  # [B]
    active = (np.arange(T)[None, :] < lengths[:, None]).astype(np.float32)  # [B,T]
    inv = 1.0 - active

    onehot = (ids[:, :, None] == np.arange(V)[None, None, :]).astype(np.float32)
    # [B,T,V] -> per-core [T,V,BL]
    whhT = np.ascontiguousarray(np.asarray(W_hh).T.astype(np.float32))  # [H,4H]
    wihT = np.ascontiguousarray(np.asarray(W_ih).T.astype(np.float32))  # [E,4H]
    embT = np.ascontiguousarray(np.asarray(emb).T.astype(np.float32))  # [E,V]
    bias2 = (np.asarray(b_ih) + np.asarray(b_hh)).astype(np.float32)  # [4H]
    dmask = np.asarray(dropout_mask).astype(np.float32)  # [B,H]

    in_maps = []
    for c in range(NCORES):
        sl = slice(c * BL, (c + 1) * BL)
        in_maps.append(
            {
                "onehot": np.ascontiguousarray(onehot[sl].transpose(1, 2, 0)),
                "whhT": whhT,
                "wihT": wihT,
                "embT": embT,
                "bias2": bias2,
                "dmask": np.ascontiguousarray(dmask[sl]),
                "bneg": np.ascontiguousarray(NEG * inv[sl]),
                "bpos": np.ascontiguousarray(-NEG * inv[sl]),
            }
        )
    return in_maps, lengths


def kernel(input_ids, emb, W_ih, W_hh, b_ih, b_hh, dropout_mask):
    from concourse.bass_utils import run_bass_kernel_spmd

    in_maps, lengths = _host_prep(
        input_ids, emb, W_ih, W_hh, b_ih, b_hh, dropout_mask
    )
    if "nc" not in _CACHE:
        _CACHE["nc"] = _build_nc()
    res = run_bass_kernel_spmd(_CACHE["nc"], in_maps, list(range(NCORES))).results

    hs = np.concatenate([r["hs"] for r in res], axis=0)  # [B,T,H]
    last_c = np.concatenate([r["lastc"] for r in res], axis=0)  # [B,H]
    last_h = hs[np.arange(B), lengths - 1, :]  # [B,H]
    return hs, last_h, last_c
